# revision 89
# baseline (speedup 1.0000x reference)
"""Trainium2 Bass kernel for 16-head causal multi-head attention.

Problem: B=2, S=2048, D=1024, H=16 (head dim 64), causal mask.
    out = softmax((XqWq+bq)(XkWk+bk)^T / 8, causal) (XvWv+bv) Wo + bo

Sharding: DP(batch=2) x TP(4 heads). Core c handles batch c//4 and heads
4*(c%4)..4*(c%4)+3: Wq/Wk/Wv column-sliced (256 cols), Wo row-sliced
(256 rows). Each core loads only its batch's X (12MB vs 24MB for pure
head-TP), computes its 4 heads end-to-end and writes a partial output
(ctx_c @ Wo_c); the host sums each batch group's 4 partials and adds
(bv @ Wo + bo).

Device-side structure (per core, 2048 tokens = 16 qtiles of 128,
processed in qtile PAIRS):
  - Scores transposed, S^T[k, q] = K @ Q^T: per (qtile-pair, ktile) a
    [128, 1024] psum tile holds both qtiles x 4 heads; ONE exp per
    (pair, ktile). Causal trim at qtile granularity; diagonal 128x128
    blocks get a tri mask on DVE after exp.
  - PV uses p as the *stationary* operand: ctx[q, d] accumulates per
    qtile in a dedicated psum bank (64 cols per head + 4 sum cols from
    N=1 ones-matmuls) - ~2x fewer PE cycles than v-stationary.
    The bank is pre-zeroed by a DVE memset; all PV matmuls accumulate
    with start=False (skip_group_check).
  - Normalization: per-partition (per-q) reciprocal+multiply on DVE;
    ctx is transposed [q,d]->[d,q] with an XBAR dma_start_transpose to
    feed the output projection.
  - Projections (q/k/v/o) are emitted as small filler closures popped
    between attention rounds so the in-order PE never starves while ACT
    runs exp.

PSUM map (8 banks): sw0/sw1 score tiles [128,1024] (2+2, alternating
ktile parity), ctx0/ctx1 (qtile parity), qko (q/k/o-proj, half-chunk
ping-pong inside the bank), aux (v-proj ping-pong).
"""

import math
from collections import deque

import numpy as np

B, S, D, H = 2, 2048, 1024, 16
DK = D // H  # 64
NCORES = 8
CPB = 4            # cores per batch group
HPC = 4            # heads per core
MD = HPC * DK      # per-core model dims (256)
P = 128
QC = 512           # tokens per chunk
SC = S             # per-core tokens (one batch)
NCH = SC // QC     # chunks (4)
KPC = QC // P      # k/q tiles per chunk (4)
NQT = SC // P      # qtiles (16)
NF = D // P        # feature tiles (8)

_PROGRAM_CACHE = {}
TRACE = False
LAST = {}
USE_XBAR = True         # XBAR dma transpose vs PE-transpose fallback
STAGE = 4               # 1=proj only, 2=+attention, 3=+norm, 4=full
SUB = 0                 # stage2 detail: 0=qk+exp, 1=+tri, 2=+pv, 3=+sums
NOFILL = False          # drain fillers before attention rounds (debug)
USE_BCAST = True       # stride-0 broadcast DVE ops


# ---------------------------------------------------------------------------
# Device program
# ---------------------------------------------------------------------------

def _mha_body(ctx, tc, io):
    from concourse import mybir
    from concourse.alu_op_type import AluOpType

    F16 = mybir.dt.float16
    F32 = mybir.dt.float32
    Exp = mybir.ActivationFunctionType.Exp

    nc = tc.nc

    xq, xk, xv = io["xq_t"], io["xk_t"], io["xv_t"]
    wq, wk, wv, wo = io["wq"], io["wk"], io["wv"], io["wo"]
    bqd, bkd = io["bq"], io["bk"]
    tri = io["tri"]
    eye = io["eye"]
    out_t = io["out_t"]

    consts = ctx.enter_context(tc.tile_pool(name="consts", bufs=1))
    xs = ctx.enter_context(tc.tile_pool(name="xs", bufs=1))
    kvs = ctx.enter_context(tc.tile_pool(name="kvs", bufs=1))
    ps = ctx.enter_context(tc.tile_pool(name="ps", bufs=1))
    cts = ctx.enter_context(tc.tile_pool(name="cts", bufs=1))
    outsp = ctx.enter_context(tc.tile_pool(name="outsp", bufs=1))
    pspool = ctx.enter_context(tc.tile_pool(name="psum", bufs=1, space="PSUM"))

    def ps_tile(tag, width=QC):
        return pspool.tile([P, width], F32, tag=tag, name=tag)

    qko_t = None  # set after pools; single version, regions round-robin
    aux_t = None

    # ---- constants (interleaved with x loads in need order) ---------------
    wq_sb = consts.tile([P, NF, MD], F16, tag="wq")
    nc.sync.dma_start(wq_sb[:], wq.rearrange("p (f m) -> p f m", m=MD))
    bq_sb = consts.tile([P, 2], F32, tag="bq")
    nc.sync.dma_start(bq_sb[:], bqd[:, :])
    bk_sb = consts.tile([P, 2], F32, tag="bk")
    nc.sync.dma_start(bk_sb[:], bkd[:, :])
    wk_sb = consts.tile([P, NF, MD], F16, tag="wk")
    wv_sb = consts.tile([P, NF, MD], F16, tag="wv")
    wo_sb = consts.tile([P, 2, D], F16, tag="wo")
    tri_sb = consts.tile([P, P], F16, tag="tri")
    eye_sb = consts.tile([P, P], F16, tag="eye")
    ones_sb = consts.tile([P, QC], F16, tag="ones")
    nc.vector.memset(ones_sb[:], 1.0)
    tri_b = tri_sb[:].rearrange("p (a q) -> p a q", a=1).broadcast_to(
        [P, HPC, P])

    # single-version psum tiles: hazards are view-overlap-tracked, so
    # disjoint regions (ping-pong halves) pipeline without version WAW
    qko_t = ps_tile("qko")
    aux_t = ps_tile("aux")
    sw_t = {0: ps_tile("sw0", width=2 * QC), 1: ps_tile("sw1", width=2 * QC)}
    ctx_t = {0: ps_tile("ctx0"), 1: ps_tile("ctx1")}
    p_t = {i: ps.tile([P, 2 * QC], F16, tag=f"p{i}", name=f"p{i}")
           for i in range(3)}
    qko_rr = {"n": 0}

    # ---- PE p-state warmup: dependency-free matmuls during initial DMAs ---
    for _ in range(10):
        nc.tensor.matmul(qko_t[:, :], ones_sb[:, 0:128],
                         ones_sb[:, :], start=True, stop=True)

    # ---- x loads: [128, 1024] half-seq tiles from SP ----------------------
    xt = {}

    def emit_x(half):
        HS = SC // 2
        for nm, src in (("q", xq), ("k", xk), ("v", xv)):
            if half == 0 and nm == "k":
                nc.scalar.dma_start(
                    wk_sb[:], wk.rearrange("p (f m) -> p f m", m=MD))
            if half == 0 and nm == "v":
                nc.scalar.dma_start(
                    wv_sb[:], wv.rearrange("p (f m) -> p f m", m=MD))
            for f in range(NF):
                t = xs.tile([P, HS], F16, tag=f"x{nm}{f}h{half}",
                            name=f"x{nm}{f}h{half}")
                nc.sync.dma_start(t[:], src[f * P:(f + 1) * P,
                                            half * HS:(half + 1) * HS])
                xt[(nm, f, half)] = t
        if half == 0:
            nc.scalar.dma_start(tri_sb[:], tri[:, :])
        else:
            nc.scalar.dma_start(wo_sb[:],
                                wo.rearrange("p (j d) -> p j d", d=D))

    emit_x(0)

    def xsl(nm, f, lo, hi):
        HS = SC // 2
        half = lo // HS
        assert (hi - 1) // HS == half
        return xt[(nm, f, half)][:, lo - half * HS:hi - half * HS]

    k_sb = kvs.tile([P, 2, SC], F16, tag="k")
    v_sb = {}    # per ktile: [128 tok, 256] (4 heads x 64)
    q_tiles = {}  # per chunk: [128, 2, 512]
    ctxT = {}    # per chunk: [128, 2, 512]
    outst = {}   # per chunk: [128, NF, 512]

    # ---- filler units -----------------------------------------------------
    def units_x(c):
        return [lambda c=c: emit_x(c)]

    def units_proj(c):
        """q/k/v projections of chunk c as filler closures."""
        c0 = c * QC
        HH = QC // 2
        qko_cell = {}

        def qk_mm(nm, w_sb, j, lo, qoff):
            qoff = (qko_rr["n"] % 2) * HH
            qko_rr["n"] += 1
            qko_cell["off"] = qoff
            for f in range(NF):
                nc.tensor.matmul(qko_t[:, qoff:qoff + HH],
                                 w_sb[:, f, j * P:(j + 1) * P],
                                 xsl(nm, f, lo, lo + HH),
                                 start=(f == 0), stop=(f == NF - 1))

        def qk_drain(nm, b_sb, j, c, hh, qoff):
            pp = qko_t
            qoff = qko_cell["off"]
            if nm == "q":
                if c not in q_tiles:
                    q_tiles[c] = ps.tile([P, 2, QC], F16, tag=f"q{c % 2}",
                                         name=f"q{c % 2}")
                dst = q_tiles[c][:, j, hh * HH:(hh + 1) * HH]
            else:
                dst = k_sb[:, j, c0 + hh * HH:c0 + (hh + 1) * HH]
            nc.vector.tensor_scalar(dst, pp[:, qoff:qoff + HH],
                                    b_sb[:, j:j + 1], None,
                                    op0=AluOpType.add)

        qk_units = {"q": [], "k": []}
        for nm, w_sb, b_sb in (("q", wq_sb, bq_sb), ("k", wk_sb, bk_sb)):
            for j in range(2):
                for hh in range(2):
                    qk_units[nm].append(
                        lambda nm=nm, w=w_sb, j=j, hh=hh:
                        qk_mm(nm, w, j, c0 + hh * HH, 0))
                    qk_units[nm].append(
                        lambda nm=nm, b=b_sb, j=j, hh=hh:
                        qk_drain(nm, b, j, c, hh, 0))

        aux_cell = {}

        def v_mm(tt):
            qoff = (tt % 2) * MD
            for f in range(NF):
                nc.tensor.matmul(aux_t[:, qoff:qoff + MD],
                                 xsl("v", f, c0 + tt * P, c0 + (tt + 1) * P),
                                 wv_sb[:, f, :],
                                 start=(f == 0), stop=(f == NF - 1))

        def v_drain(tt, kt):
            pp = aux_t
            qoff = (tt % 2) * MD
            vt = kvs.tile([P, HPC, DK + 1], F16, tag=f"v{kt}", name=f"v{kt}")
            nc.vector.memset(vt[:, :, DK:DK + 1], 1.0)
            nc.vector.tensor_copy(
                vt[:, :, 0:DK],
                pp[:, qoff:qoff + MD].rearrange("p (h d) -> p h d", d=DK))
            v_sb[kt] = vt

        v_units = []
        for tt in range(KPC):
            v_units.append(lambda tt=tt: v_mm(tt))
            v_units.append(lambda tt=tt, kt=c * KPC + tt: v_drain(tt, kt))
        return qk_units["q"], qk_units["k"], v_units

    def units_oproj(c):
        """output projection of chunk c (+ its out DMA), split by hh half:
        hh=0 reads ctxT qtiles 4c,4c+1 (ready after the chunk's first pair),
        hh=1 reads qtiles 4c+2,4c+3."""
        out = {0: [], 1: []}
        cell = {}
        last = (c == NCH - 1)

        def alloc():
            outst[c] = outsp.tile([P, NF, QC], F16, tag=f"o{c % 2}",
                                  name=f"o{c % 2}")
        out[0].insert(0, alloc)

        def o_mm(mt, hh):
            HH = QC // 2
            nslot = 4 if last else 2
            slot = qko_rr["n"] % nslot
            qko_rr["n"] += 1
            cell["slot"] = slot
            bank = aux_t if slot >= 2 else qko_t
            qoff = (slot % 2) * HH
            ct = ctxT[c]
            for j in range(2):
                nc.tensor.matmul(bank[:, qoff:qoff + HH],
                                 wo_sb[:, j, mt * P:(mt + 1) * P],
                                 ct[:, j, hh * HH:(hh + 1) * HH],
                                 start=(j == 0), stop=(j == 1))

        def o_drain(mt, hh, k):
            HH = QC // 2
            slot = cell["slot"]
            bank = aux_t if slot >= 2 else qko_t
            qoff = (slot % 2) * HH
            dst = outst[c][:, mt, hh * HH:(hh + 1) * HH]
            if k % 2 == 1 and c < 0:
                nc.scalar.copy(dst, bank[:, qoff:qoff + HH])
            else:
                nc.vector.tensor_copy(dst, bank[:, qoff:qoff + HH])

        k = 0
        if last:
            # quarter granularity: qt-quarter q=2 usable after norm(qp,0) of
            # the final pair; q=3 after norm(qp,1)
            def o_mm_q(mt, q):
                HQ = P
                nslot = 4
                slot = qko_rr["n"] % nslot
                qko_rr["n"] += 1
                cell["slot"] = slot
                bank = aux_t if slot >= 2 else qko_t
                qoff = (slot % 2) * (QC // 2)
                ct = ctxT[c]
                for j in range(2):
                    nc.tensor.matmul(bank[:, qoff:qoff + HQ],
                                     wo_sb[:, j, mt * P:(mt + 1) * P],
                                     ct[:, j, q * HQ:(q + 1) * HQ],
                                     start=(j == 0), stop=(j == 1))

            def o_drain_q(mt, q):
                HQ = P
                slot = cell["slot"]
                bank = aux_t if slot >= 2 else qko_t
                qoff = (slot % 2) * (QC // 2)
                dst = outst[c][:, mt, q * HQ:(q + 1) * HQ]
                nc.vector.tensor_copy(dst, bank[:, qoff:qoff + HQ])

            for hh in range(2):
                for q in (2 * hh, 2 * hh + 1):
                    for mt in range(NF):
                        out[hh].append(lambda mt=mt, q=q: o_mm_q(mt, q))
                        out[hh].append(lambda mt=mt, q=q: o_drain_q(mt, q))
        else:
            for hh in range(2):
                for mt in range(NF):
                    out[hh].append(lambda mt=mt, hh=hh: o_mm(mt, hh))
                    out[hh].append(lambda mt=mt, hh=hh, k=k: o_drain(mt, hh, k))
                    k += 1

        def o_dma(hh):
            HH = QC // 2
            nc.gpsimd.dma_start(
                out_t.rearrange("(m p) n -> p m n", p=P)[
                    :, :, c * QC + hh * HH:c * QC + (hh + 1) * HH],
                outst[c][:, :, hh * HH:(hh + 1) * HH])

        def o_dma_q(q):
            nc.gpsimd.dma_start(
                out_t.rearrange("(m p) n -> p m n", p=P)[
                    :, :, c * QC + q * P:c * QC + (q + 1) * P],
                outst[c][:, :, q * P:(q + 1) * P])
        if last:
            # fire the q14 piece as soon as its drains land; only the q15
            # quarter remains on the critical tail
            out[1].insert(len(out[1]) - 16, lambda: o_dma_q(2))
            out[1].append(lambda: o_dma_q(3))
            out[0].append(lambda: o_dma(0))
        else:
            out[0].append(lambda: o_dma(0))
            out[1].append(lambda: o_dma(1))
        return out[0], out[1]

    # ---- attention --------------------------------------------------------
    fillers = deque()

    def pop_fillers(n):
        for _ in range(n):
            if fillers:
                fillers.popleft()()

    ctx_ps = {}

    def emit_ctx_memset(qip):
        cp = ctx_t[qip]
        ctx_ps[qip] = cp
        nc.vector.memset(cp[:, 0:HPC * (DK + 1)], 0.0)

    def emit_qk_exp(qp, kt):
        """QK + exp for (pair qp, ktile kt). Returns PV closure."""
        gq0 = 2 * qp            # even qtile
        c = gq0 // KPC          # chunk of the pair
        qtile = q_tiles[c]
        tail = (kt == gq0 + 1)  # odd-qtile-only round
        sw = sw_t[kt % 2]
        pt = p_t[kt % 3]
        kcols = slice(kt * P, kt * P + P)

        def kap(h):
            return k_sb[(h % 2) * 64:(h % 2) * 64 + 64, h // 2, kcols]

        def qap(h, gqt):
            co = (gqt % KPC) * P
            return qtile[(h % 2) * 64:(h % 2) * 64 + 64, h // 2, co:co + P]

        qips = (1,) if tail else (0, 1)

        def scol(h, qip):
            return (h % 2) * QC + (h // 2) * 2 * P + qip * P

        for h in (0, 2, 1, 3):  # base-0 heads first, then base-64
            if tail:
                col = scol(h, 1)
                nc.tensor.matmul(sw[:, col:col + P],
                                 kap(h), qap(h, gq0 + 1),
                                 start=True, stop=True)
            else:
                co0 = (gq0 % KPC) * P
                col = scol(h, 0)
                nc.tensor.matmul(
                    sw[:, col:col + 2 * P], kap(h),
                    qtile[(h % 2) * 64:(h % 2) * 64 + 64, h // 2,
                          co0:co0 + 2 * P],
                    start=True, stop=True)
        if tail:
            # odd-qip blocks: offsets 128,384,640,896 -> strided view
            swv = sw[:].rearrange("p (b t q) -> p b t q", t=2, q=P)[:, :, 1, :]
            ptv = pt[:].rearrange("p (b t q) -> p b t q", t=2, q=P)[:, :, 1, :]
            nc.scalar.activation(ptv, swv, Exp, scale=0.125)
        else:
            nc.scalar.activation(pt[:], sw[:], Exp, scale=0.125)
        for dq, is_diag in ((0, kt == gq0), (1, tail)):
            if not is_diag:
                continue
            ptv = pt[:].rearrange("p (b t q) -> p b t q", t=2, q=P)[:, :, dq, :]
            if USE_BCAST:
                nc.vector.tensor_tensor(ptv, ptv, tri_b, op=AluOpType.mult)
            else:
                for b in range(4):
                    col = b * 2 * P + dq * P
                    nc.vector.tensor_mul(pt[:, col:col + P],
                                         pt[:, col:col + P], tri_sb[:])

        def emit_pv():
            if STAGE == 2 and SUB < 2:
                return
            for qip in qips:
                cp = ctx_ps[qip]
                for h in range(HPC):
                    col = (h % 2) * QC + (h // 2) * 2 * P + qip * P
                    psl = pt[:, col:col + P]
                    nc.tensor.matmul(cp[:, h * (DK + 1):(h + 1) * (DK + 1)],
                                     psl, v_sb[kt][:, h, 0:DK + 1],
                                     start=False, stop=False,
                                     skip_group_check=True)
        return emit_pv

    def emit_norm(qp, qip):
        """normalize + transpose qtile 2*qp+qip."""
        gqt = 2 * qp + qip
        c = gqt // KPC
        cp = ctx_ps[qip]
        cpv = cp[:, 0:HPC * (DK + 1)].rearrange("p (h d) -> p h d", d=DK + 1)
        rc = cts.tile([P, HPC], F32, tag=f"rc{qip}", name=f"rc{qip}")
        nc.vector.reciprocal(
            rc[:].rearrange("p (h a) -> p h a", a=1), cpv[:, :, DK:DK + 1])
        csb = cts.tile([P, MD], F16, tag=f"csb{qip}", name=f"csb{qip}")
        if USE_BCAST:
            nc.vector.tensor_tensor(
                csb[:].rearrange("p (h d) -> p h d", d=DK),
                cpv[:, :, 0:DK],
                rc[:].rearrange("p (h a) -> p h a", a=1).broadcast_to(
                    [P, HPC, DK]),
                op=AluOpType.mult)
        else:
            for h in range(HPC):
                nc.vector.tensor_scalar(
                    csb[:, h * DK:(h + 1) * DK],
                    cp[:, h * DK:(h + 1) * DK],
                    rc[:, h:h + 1], None, op0=AluOpType.mult)
        if c not in ctxT:
            ctxT[c] = cts.tile([P, 2, QC], F16, tag=f"ctxT{c % 2}",
                               name=f"ctxT{c % 2}")
        co = (gqt % KPC) * P
        if USE_XBAR:
            nc.sync.dma_start_transpose(ctxT[c][:, :, co:co + P], csb[:])
        else:
            # PE transpose into spare f16-bitcast space of the ctx bank
            tp = cp[:, 320:448].bitcast(F16)  # [128, 256] f16 scratch
            for j in range(2):
                nc.tensor.matmul(tp[:, j * P:(j + 1) * P],
                                 csb[:, j * P:(j + 1) * P], eye_sb[:],
                                 is_transpose=True, start=True, stop=True)
                nc.vector.tensor_copy(ctxT[c][:, j, co:co + P],
                                      tp[:, j * P:(j + 1) * P])

    # ---- main schedule ----------------------------------------------------
    emit_x(1)

    wfill_n = {"i": 0}

    def wfill(n):
        """dummy matmuls into the sw banks (QK-compatible (64,128) config)
        to keep PE busy while x tiles arrive."""
        for _ in range(n):
            b = wfill_n["i"] % 2
            wfill_n["i"] += 1
            nc.tensor.matmul(sw_t[b][0:128, 0:QC], ones_sb[0:64, 0:128],
                             ones_sb[0:64, 0:QC], start=True, stop=True)

    q0u, k0u, v0u = units_proj(0)
    q1u, k1u, v1u = units_proj(1)
    for i, u in enumerate(q0u + q1u + k0u + k1u + v0u):
        u()
        if i % 2 == 1:
            wfill(2)
    fillers.extend(v1u)

    if STAGE == 1:
        for c in range(2, NCH):
            qu, ku, vu = units_proj(c)
            fillers.extend(qu + ku + vu)
        while fillers:
            fillers.popleft()()
    else:
        oproj_h = {}
        pair_seq = []
        for c in range(NCH):
            pair_seq += [(c, 2 * c, 0), (c, 2 * c + 1, 1)]
        for c, qp, idx in pair_seq:
            pending_prev = None
            if idx == 0 and c > 0:  # chunk start
                if STAGE >= 4:
                    pending_prev = oproj_h[c - 1][1]
                if c + 1 < NCH:
                    qu, ku, vu = units_proj(c + 1)
                    fillers.extend(qu + ku + vu)
            pending_oproj = (idx == 1 and STAGE >= 4)
            rounds_left = (2 * qp + 2)
            if NOFILL:
                while fillers:
                    fillers.popleft()()
            if qp == 0:
                emit_ctx_memset(0)
                emit_ctx_memset(1)
            pv_q = deque()
            for kt in range(2 * qp + 2):
                if kt == (2 if qp < 5 else 3) and pending_prev:
                    fillers.extend(pending_prev)
                    pending_prev = None
                if kt == (6 if qp == 5 else (5 if qp == 7 else 2)) and pending_oproj:
                    # delayed so the prior norms' transposes land first
                    oproj_h[c] = units_oproj(c)
                    fillers.extend(oproj_h[c][0])
                    pending_oproj = False
                # fillers first: their DVE drains queue ahead of this
                # round's exp-gated tri-mask on the in-order DVE
                n = max(1, -(-len(fillers) // max(rounds_left + 4, 1)))
                pop_fillers(min(n, 3))
                pv_q.append(emit_qk_exp(qp, kt))
                if len(pv_q) > 2:  # PV lags exp by two rounds
                    pv_q.popleft()()
                rounds_left -= 1
            while len(pv_q) > 2:
                pv_q.popleft()()
            pv_q.popleft()()
            if STAGE >= 3:
                emit_norm(qp, 0)
                emit_ctx_memset(0)
                if STAGE >= 4 and c == NCH - 1 and idx == 1:
                    fillers.extend(oproj_h[c][1][:17])
                    oproj_h[c] = (oproj_h[c][0], oproj_h[c][1][17:])
            pv_q.popleft()()
            if STAGE >= 3:
                emit_norm(qp, 1)
                if qp < NQT // 2 - 1:
                    emit_ctx_memset(1)

        while fillers:
            fillers.popleft()()
        if STAGE >= 4:
            for u in oproj_h[NCH - 1][1]:
                u()

    if STAGE < 4:
        # dummy output write so the program has its ExternalOutput
        dummy = outsp.tile([P, NF, QC], F16, tag="dummy", name="dummy")
        nc.vector.memset(dummy[:], 0.0)
        nc.gpsimd.dma_start(
            out_t.rearrange("(m p) n -> p m n", p=P)[:, :, 0:QC], dummy[:])


def build_program():
    import concourse.tile as tile
    from concourse import bacc, mybir
    from contextlib import ExitStack

    F16 = mybir.dt.float16
    F32 = mybir.dt.float32

    nc = bacc.Bacc("TRN2", target_bir_lowering=False, debug=False)
    io = {
        "xq_t": nc.dram_tensor("xq_t", [D, SC], F16, kind="ExternalInput").ap(),
        "xk_t": nc.dram_tensor("xk_t", [D, SC], F16, kind="ExternalInput").ap(),
        "xv_t": nc.dram_tensor("xv_t", [D, SC], F16, kind="ExternalInput").ap(),
        "wq": nc.dram_tensor("wq", [P, NF * MD], F16, kind="ExternalInput").ap(),
        "wk": nc.dram_tensor("wk", [P, NF * MD], F16, kind="ExternalInput").ap(),
        "wv": nc.dram_tensor("wv", [P, NF * MD], F16, kind="ExternalInput").ap(),
        "wo": nc.dram_tensor("wo", [P, 2 * D], F16, kind="ExternalInput").ap(),
        "bq": nc.dram_tensor("bq", [P, 2], F32, kind="ExternalInput").ap(),
        "bk": nc.dram_tensor("bk", [P, 2], F32, kind="ExternalInput").ap(),
        "tri": nc.dram_tensor("tri", [P, P], F16, kind="ExternalInput").ap(),
        "eye": nc.dram_tensor("eye", [P, P], F16, kind="ExternalInput").ap(),
        "out_t": nc.dram_tensor("out_t", [D, SC], F16, kind="ExternalOutput").ap(),
    }
    with tile.TileContext(nc) as tc, ExitStack() as ctx:
        _mha_body(ctx, tc, io)
    nc.compile()
    return nc


# ---------------------------------------------------------------------------
# Host side
# ---------------------------------------------------------------------------

def _np_reference(query, key, value, mask, Wq, bq, Wk, bk, Wv, bv, Wo, bo):
    q = (query.reshape(-1, D) @ Wq + bq).reshape(B, S, H, DK).transpose(0, 2, 1, 3)
    k = (key.reshape(-1, D) @ Wk + bk).reshape(B, S, H, DK).transpose(0, 2, 1, 3)
    v = (value.reshape(-1, D) @ Wv + bv).reshape(B, S, H, DK).transpose(0, 2, 1, 3)
    scores = np.einsum("bhqd,bhkd->bhqk", q, k) / math.sqrt(DK)
    scores = np.where(mask[:, None, :, :] == 0, np.float32(-1e9), scores)
    scores -= scores.max(axis=-1, keepdims=True)
    p = np.exp(scores)
    p /= p.sum(axis=-1, keepdims=True)
    x = np.einsum("bhqk,bhkd->bhqd", p, v)
    x = x.transpose(0, 2, 1, 3).reshape(B, -1, D)
    return (x @ Wo + bo).astype(np.float32)


def _wlayout(w):
    """[D, M] -> [128, (D//128)*M] fp16 partition-major:
    out[p, f*M + m] = w[f*128 + p, m]."""
    d = w.shape[0]
    nf = d // P
    return np.ascontiguousarray(
        w.reshape(nf, P, -1).transpose(1, 0, 2).reshape(P, -1)).astype(np.float16)


def _shard_inputs(query, key, value, Wq, bq, Wk, bk, Wv, Wo):
    f16 = np.float16
    idx = np.arange(P)
    tri = (idx[:, None] <= idx[None, :]).astype(f16)  # tri[k, q] = k <= q
    xts = []
    for b in range(B):
        xts.append((
            np.ascontiguousarray(query[b].T).astype(f16),
            np.ascontiguousarray(key[b].T).astype(f16),
            np.ascontiguousarray(value[b].T).astype(f16),
        ))
    in_maps = []
    for c in range(NCORES):
        bb, hg = c // CPB, c % CPB
        sl = slice(hg * MD, (hg + 1) * MD)
        xq_t, xk_t, xv_t = xts[bb]
        in_maps.append({
            "xq_t": xq_t,
            "xk_t": xk_t,
            "xv_t": xv_t,
            "wq": _wlayout(Wq[:, sl]),
            "wk": _wlayout(Wk[:, sl]),
            "wv": _wlayout(Wv[:, sl]),
            "wo": _wlayout(Wo[sl, :]),
            "bq": np.ascontiguousarray(
                bq[sl].reshape(2, P).T).astype(np.float32),
            "bk": np.ascontiguousarray(
                bk[sl].reshape(2, P).T).astype(np.float32),
            "tri": tri,
            "eye": np.eye(P, dtype=f16),
        })
    return in_maps


def kernel(**inputs):
    query = np.asarray(inputs["query"], np.float32)
    key = np.asarray(inputs["key"], np.float32)
    value = np.asarray(inputs["value"], np.float32)
    mask = np.asarray(inputs["mask"])
    Wq = np.asarray(inputs["Wq"], np.float32)
    bq = np.asarray(inputs["bq"], np.float32)
    Wk = np.asarray(inputs["Wk"], np.float32)
    bk = np.asarray(inputs["bk"], np.float32)
    Wv = np.asarray(inputs["Wv"], np.float32)
    bv = np.asarray(inputs["bv"], np.float32)
    Wo = np.asarray(inputs["Wo"], np.float32)
    bo = np.asarray(inputs["bo"], np.float32)

    tril = np.tril(np.ones((S, S), np.int8))
    if mask.shape != (B, S, S) or not np.array_equal(
            (mask != 0).astype(np.int8), np.broadcast_to(tril, (B, S, S))):
        return _np_reference(query, key, value, mask,
                             Wq, bq, Wk, bk, Wv, bv, Wo, bo)

    in_maps = _shard_inputs(query, key, value, Wq, bq, Wk, bk, Wv, Wo)
    outs = _run_spmd(in_maps)  # [8, D, SC]

    const = (bv @ Wo + bo)[None, :]
    res = np.empty((B, S, D), np.float32)
    for b in range(B):
        acc = outs[b * CPB:(b + 1) * CPB].astype(np.float32).sum(axis=0)
        res[b] = acc.T + const
    return res


def _get_exec():
    if "exec" in _PROGRAM_CACHE:
        return _PROGRAM_CACHE["exec"]
    import jax
    from jax.sharding import Mesh, PartitionSpec
    from jax.experimental.shard_map import shard_map
    import concourse.mybir as mybir
    from concourse import bass2jax

    nc = build_program()
    _PROGRAM_CACHE["nc"] = nc
    bass2jax.install_neuronx_cc_hook()
    partition_name = nc.partition_id_tensor.name if nc.partition_id_tensor else None
    in_names, out_names, out_avals, zero_outs = [], [], [], []
    for alloc in nc.m.functions[0].allocations:
        if not isinstance(alloc, mybir.MemoryLocationSet):
            continue
        name = alloc.memorylocations[0].name
        if alloc.kind == "ExternalInput":
            if name != partition_name:
                in_names.append(name)
        elif alloc.kind == "ExternalOutput":
            out_names.append(name)
            shape = tuple(alloc.tensor_shape)
            dtype = mybir.dt.np(alloc.dtype)
            out_avals.append(jax.core.ShapedArray(shape, dtype))
            zero_outs.append(np.zeros(shape, dtype))
    n_params = len(in_names)
    all_in_names = list(in_names) + list(out_names)
    if partition_name is not None:
        all_in_names.append(partition_name)

    def _body(*args):
        operands = list(args)
        if partition_name is not None:
            operands.append(bass2jax.partition_id_tensor())
        return tuple(bass2jax._bass_exec_p.bind(
            *operands,
            out_avals=tuple(out_avals),
            in_names=tuple(all_in_names),
            out_names=tuple(out_names),
            lowering_input_output_aliases=(),
            sim_require_finite=True,
            sim_require_nnan=True,
            nc=nc,
        ))

    devices = jax.devices()[:NCORES]
    assert len(devices) >= NCORES, f"need {NCORES} neuron cores"
    mesh = Mesh(np.asarray(devices[:NCORES]), ("core",))
    fn = jax.jit(
        shard_map(_body, mesh=mesh,
                  in_specs=(PartitionSpec("core"),) * (n_params + len(zero_outs)),
                  out_specs=(PartitionSpec("core"),) * len(out_names),
                  check_rep=False),
        donate_argnums=tuple(range(n_params, n_params + len(out_names))),
        keep_unused=True)
    _PROGRAM_CACHE["exec"] = (fn, in_names, zero_outs)
    return _PROGRAM_CACHE["exec"]


def _run_spmd(in_maps):
    fn, in_names, zero_outs = _get_exec()
    concat_in = [np.concatenate([np.asarray(in_maps[c][nm])
                                 for c in range(NCORES)], axis=0)
                 for nm in in_names]
    concat_zero = [np.zeros((NCORES * z.shape[0], *z.shape[1:]), z.dtype)
                   for z in zero_outs]
    out = fn(*concat_in, *concat_zero)
    LAST["out"] = out
    return np.asarray(out[0]).reshape(NCORES, D, SC)
